# revision 17
# baseline (speedup 1.0000x reference)
"""ColBERT MaxSim kernel for 8 Trainium2 NeuronCores.

scores[b, c] = sum_n max_s (qs[b, n, :] . ps[c, s, :])
  qs: (64, 32, 128) f32, ps: (64, 1024, 128) f32 -> scores: (64, 64) f32

Sharding: docs (c) are sharded 8 per core; qs is replicated. Each core
computes its (64, 8) score tile; the host concatenates along c. This puts
only ~2.5 MiB of DMA on each core (vs 33 MiB for batch-sharding) while the
per-core compute volume is identical.

Per-core dataflow (mode "pair", the default):
  - fp32 matmul on TRN2 streams at 2 cyc/col x 2 passes = 4x slower than
    16-bit, so inputs are cast to fp16 on the host (10 mantissa bits;
    measured end-to-end rel err ~2.4e-5 vs the fp32 reference).
  - Doc tokens are combined in PAIRS on the host: P+ = (P_even+P_odd)/2,
    P- = (P_even-P_odd)/2. Using max(a,b) = (a+b)/2 + |a-b|/2:
      PE:  S = Q.P+  (one PSUM bank),  D = Q.P-  (second bank)
      ACT: A = |D|   (pointwise Abs, PSUM->SBUF fp16, the only non-DVE
                      engine that can read PSUM)
      PE:  S += I.A  (accumulating identity matmul, start=False) — S's
                      bank now holds the 512 per-pair maxes
      DVE: reduce_max over [128, 2, 512] (two docs per op) -> maxcols
    This halves the VectorE reduce volume. The steady state is a genuine
    three-way fixed point at ~1.39us per 2-doc iteration: ACT is busy
    1.40us (2 ABS at (512+352)/1.2GHz), PE 1.32us (6 matmuls, LDWEIGHTS
    hidden by the reorder window), and the S-bank round trip
    (fill -> abs -> ident -> reduce, ~2.8us over 2 PSUM bufs) also lands
    at 1.4us. Batching/reordering any one leg pushes another past it, so
    the remaining headroom is all in startup and tail:
      * first DMA chunks are per-doc and split across both HWDGE queues
        (sync + scalar) so the first real matmul can start as soon as
        qsT-mg01 + psP/psM-doc0 land (~290 KiB critical bytes);
      * `ones` (block-diag) and `ident` are synthesized on-chip (gpsimd
        memset / affine_select), keeping the DMA queues clear;
      * the final token-sum matmul is split in half: docs 0-3 are summed
        and DMA'd out mid-kernel (hiding that DMA's ~1.5us latency), and
        the output DRAM layout is [q, half, mg, d] so each half is one
        contiguous 1 KiB burst instead of 65 strided 32 B packets.
  - Token-sum over each query's 32 rows via small fp16 matmuls with a
    block-diagonal ones matrix: out[4, 16, 4] = onesT.T @ maxcols-half.

Mode "direct" (env KERNEL_MODE=direct) is the exact-fp32 fallback:
fp32 matmuls + VectorE reduce_max straight from PSUM (~2.6x slower).
"""

import os
import sys
from contextlib import ExitStack

import numpy as np

sys.path.insert(0, "/opt/trn_rl_repo")
sys.path.insert(0, "/opt/trn_rl_repo/concourse")

import bass_rust
import concourse.bass as bass
import concourse.mybir as mybir
import concourse.tile as tile
from concourse import bass_utils

# Problem shape (hardcoded per contract)
N_CORES = 8
NQ, TQ, D = 64, 32, 128          # queries, query tokens, dim
ND, TD = 64, 1024                # docs, doc tokens
DOCS_PER_CORE = ND // N_CORES    # 8
QROWS = NQ * TQ                  # 2048 query-token rows
MG = QROWS // 128                # 16 M-groups of 128 rows
QPG = 128 // TQ                  # 4 queries per M-group
NPAIR = TD // 2                  # 512 token pairs per doc
HALF_D = DOCS_PER_CORE // 2      # 4 docs per output half

F32 = mybir.dt.float32
F16 = mybir.dt.float16

MODE = os.environ.get("KERNEL_MODE", "pair")


def _split_multi_waits(nc):
    """This walrus build rejects >1 embedded sync wait per instruction
    ("Too many sync wait commands"). Split extras onto single-wait NoOps
    inserted just before the instruction on the same engine — semantically
    identical (per-engine program order is preserved)."""
    n_split = 0
    for fn in nc.m.functions:
        for blk in fn.blocks:
            out = []
            for ins in blk.instructions:
                si = ins.sync_info
                waits = list(si.on_wait) if si and si.on_wait else []
                if len(waits) > 1:
                    for j, w in enumerate(waits[:-1]):
                        nop = mybir.InstNoOp(
                            name=f"{ins.name}_sw{j}", ins=[], outs=[])
                        nop.engine = ins.engine
                        nop.sync_info = bass_rust.SyncInfo(
                            on_wait=[w], on_update=[])
                        out.append(nop)
                    ins.sync_info = bass_rust.SyncInfo(
                        on_wait=[waits[-1]], on_update=list(si.on_update))
                    n_split += 1
                out.append(ins)
            blk.instructions = out
    return n_split


def _build_pair_module():
    nc = bass.Bass("TRN2", target_bir_lowering=False, debug=False)

    qsT = nc.dram_tensor("qsT", [D, QROWS], F16, kind="ExternalInput").ap()
    psP = nc.dram_tensor("psP", [D, DOCS_PER_CORE * NPAIR], F16,
                         kind="ExternalInput").ap()
    psM = nc.dram_tensor("psM", [D, DOCS_PER_CORE * NPAIR], F16,
                         kind="ExternalInput").ap()
    # out[q, half, mg, dl] = scores[(mg*4 + q), half*4 + dl] — each half is
    # a contiguous 1 KiB per-partition burst for the output DMA.
    out = nc.dram_tensor("out", [QPG, 2, MG, HALF_D], F32,
                         kind="ExternalOutput").ap()

    with tile.TileContext(nc) as tc, ExitStack() as ctx:
        const = ctx.enter_context(tc.tile_pool(name="const", bufs=1))
        stage = ctx.enter_context(tc.tile_pool(name="stage", bufs=10))
        # PSUM plan (8 banks): 2 × 2-bank S tiles + 4 × 1-bank D tiles.
        # (3 S bufs + 2 D bufs measured 18% SLOWER: with only two D banks
        # every D matmul chains directly onto the ABS two slots back, and
        # the pipeline locks at ~1.58us/iter with ~830ns PE waits.)
        psumS = ctx.enter_context(
            tc.tile_pool(name="psumS", bufs=2, space="PSUM"))
        psumD = ctx.enter_context(
            tc.tile_pool(name="psumD", bufs=4, space="PSUM"))

        # DMA issue costs ~650ns per dma_start on a sequencer; data lands
        # ~4us after issue (fixed DMA round trip under 8-core contention).
        # Critical-path bytes are minimized: the first real matmul needs
        # only qsT mg0 (32 KiB, sync) + psP doc0 (128 KiB, sync) +
        # psM doc0 (128 KiB, scalar). Everything else queues behind.
        qsT_sb = const.tile([D, QROWS], F16)
        psP_sb = const.tile([D, DOCS_PER_CORE * NPAIR], F16)
        psM_sb = const.tile([D, DOCS_PER_CORE * NPAIR], F16)
        # The first descriptor on each queue pays ~2.5us of fixed DMA
        # startup latency; everything behind it pipelines at full rate.
        # So the first chunks are as small as a whole consumer needs:
        # qsT M-group 0 alone (32 KB) unblocks the gated warmups and the
        # first S matmul ~0.5us earlier than a 64 KB chunk would.
        q0 = 128        # qsT M-group 0 (32 KB)
        c0 = 2 * NPAIR  # docs 0-1 of psP/psM (256 KB), split per-doc
        nc.sync.dma_start(qsT_sb[:, 0:q0], qsT[:, 0:q0])
        nc.scalar.dma_start(psM_sb[:, 0:NPAIR], psM[:, 0:NPAIR])
        nc.sync.dma_start(psP_sb[:, 0:NPAIR], psP[:, 0:NPAIR])
        nc.scalar.dma_start(psM_sb[:, NPAIR:c0], psM[:, NPAIR:c0])
        nc.sync.dma_start(psP_sb[:, NPAIR:c0], psP[:, NPAIR:c0])
        nc.sync.dma_start(qsT_sb[:, q0:2 * q0], qsT[:, q0:2 * q0])
        # Prefetch the Abs ACT table set (~2.7us TABLE_LOAD + drain) NOW —
        # emitted here it overlaps the initial DMA transfers instead of
        # gating the first real abs.
        warm = stage.tile([1, 2], F16, tag="warm")
        nc.gpsimd.memset(warm[:], 0.0)
        warm2 = stage.tile([1, 2], F16, tag="warm2")
        nc.scalar.activation(warm2[:], warm[:],
                             mybir.ActivationFunctionType.Abs)
        # qsT M-groups 2-3 go as their own small chunk ahead of the 384 KiB
        # remainder: iters 0-1 run off mg0-1 while this lands (~14us), so
        # M-group 2 no longer stalls ~1.3us on the big chunk (~17us).
        nc.sync.dma_start(qsT_sb[:, 2 * q0:4 * q0], qsT[:, 2 * q0:4 * q0])
        nc.sync.dma_start(qsT_sb[:, 4 * q0:], qsT[:, 4 * q0:])
        nc.scalar.dma_start(psM_sb[:, c0:], psM[:, c0:])
        nc.sync.dma_start(psP_sb[:, c0:], psP[:, c0:])

        # ident / ones are synthesized on-chip (gpsimd is otherwise idle),
        # keeping the HWDGE queues free for the real input tensors.
        ident_src = const.tile([128, 128], F16)
        nc.gpsimd.memset(ident_src[:], 1.0)
        ident_sb = const.tile([128, 128], F16)
        nc.gpsimd.affine_select(
            ident_sb[:], ident_src[:], pattern=[[-1, 128]],
            compare_op=mybir.AluOpType.is_equal, fill=0.0,
            base=0, channel_multiplier=1)
        ones_sb = const.tile([128, QPG], F16)
        nc.gpsimd.memset(ones_sb[:], 0.0)
        for q in range(QPG):
            nc.gpsimd.memset(ones_sb[q * TQ:(q + 1) * TQ, q:q + 1], 1.0)

        # HAM warmup: the PE is otherwise idle from the end of the NEFF
        # preamble (~7.5us) until the first DMA chunks land (~12us), and the
        # HAM clock gate needs ~3.4us of sustained PE activity to lift the
        # throttle from 1.2 to 2.4 GHz. A burst of matmuls on an
        # uninitialized (never-read) tile fills the activity window for
        # free, so the real matmul stream starts warm. 8 ungated (~3.4us
        # cold) + 2 gated on the first qsT chunk bridge the remaining gap
        # on cores whose DMA lands late (8-core HBM contention skews
        # arrivals by ~2.5us).
        garbage = const.tile([128, NPAIR], F16)
        nc.gpsimd.memset(garbage[:], 0.0)
        for _ in range(8):
            wt = psumD.tile([128, NPAIR], F32, tag="d")
            nc.tensor.matmul(wt[:], lhsT=garbage[:, 0:128], rhs=garbage[:],
                             start=True, stop=True)
        # The gated pair is narrow (N=128): their job is a blip of PE
        # activity after the first qsT chunk lands, not work — at N=512
        # they queue 0.85us of matmul ahead of the first real S matmul.
        for _ in range(2):
            wt = psumD.tile([128, NPAIR], F32, tag="d")
            nc.tensor.matmul(wt[:, 0:128], lhsT=qsT_sb[:, 0:128],
                             rhs=garbage[:, 0:128], start=True, stop=True)

        # maxcols[p, mg, dloc] = max over doc dloc's tokens for row p of mg
        maxcols = const.tile([128, MG, DOCS_PER_CORE], F16)
        out_sb = const.tile([QPG, 2, MG * HALF_D], F32)

        def emit_fin(half):
            # Token-sum for docs [half*4, half*4+4): fin[q, mg, dl] =
            # sum_p ones[p, q] * maxcols[p, mg, half*4+dl]. Uses a psumD
            # rotation slot (the fin tile is tiny and its deps are long
            # done, so the extra rotation costs ~0.4us of PE time at most).
            finh = psumD.tile([QPG, MG * HALF_D], F32, tag="d")
            src = maxcols[:, :, half * HALF_D:(half + 1) * HALF_D]
            nc.tensor.matmul(finh[:].rearrange("q (mg d) -> q mg d",
                                               d=HALF_D),
                             lhsT=ones_sb[:], rhs=src,
                             start=True, stop=True)
            nc.vector.tensor_copy(out_sb[:, half], finh[:])
            nc.sync.dma_start(
                out.rearrange("q h mg d -> q h (mg d)")[:, half],
                out_sb[:, half])

        for dp in range(DOCS_PER_CORE // 2):
            for mg in range(MG):
                lhsT = qsT_sb[:, mg * 128:(mg + 1) * 128]
                # Two docs (2*dp, 2*dp+1) share one 2-bank S tile so the
                # VectorE reduce below covers both in a single instruction.
                s2 = psumS.tile([128, 2 * NPAIR], F32, tag="s")
                for h in range(2):
                    dloc = 2 * dp + h
                    sl = slice(dloc * NPAIR, (dloc + 1) * NPAIR)
                    sb = s2[:, h * NPAIR:(h + 1) * NPAIR]
                    # S = Q.P+ (accumulation group stays open)
                    nc.tensor.matmul(sb, lhsT=lhsT,
                                     rhs=psP_sb[:, sl], start=True,
                                     stop=False, skip_group_check=True)
                    # D = Q.P- (separate pool: released after ACT)
                    dt = psumD.tile([128, NPAIR], F32, tag="d")
                    nc.tensor.matmul(dt[:], lhsT=lhsT,
                                     rhs=psM_sb[:, sl], start=True,
                                     stop=True, skip_group_check=True)
                    # A = |D| (fp16, SBUF) on ScalarE — the 2nd PSUM reader
                    a = stage.tile([128, NPAIR], F16)
                    nc.scalar.activation(a[:], dt[:],
                                         mybir.ActivationFunctionType.Abs)
                    # S += I.A  -> S half now holds the 512 per-pair maxes
                    nc.tensor.matmul(sb, lhsT=ident_sb[:],
                                     rhs=a[:], start=False, stop=True,
                                     skip_group_check=True)
                # One [128, 2, 512] reduce for both docs: two per-doc
                # [1, 512] reduces measured 693ns each (DVE 1386/iter +
                # sems -> 1572ns/iter pace, 118.6us total) vs 1224ns
                # batched — the per-instruction overhead swamps the
                # earlier S-bank release.
                nc.vector.reduce_max(
                    maxcols[:, mg, 2 * dp:2 * dp + 2],
                    s2[:].rearrange("p (h n) -> p h n", h=2),
                    axis=mybir.AxisListType.X)
                if dp == 2 and mg == 4:
                    # Docs 0-3 finished ~1.4us/iter * 5 iters ago; their
                    # token-sum + output DMA hide entirely mid-kernel.
                    emit_fin(0)
        emit_fin(1)

    return nc


def _build_direct_module():
    """Exact-fp32 fallback: fp32 matmuls + DVE reduce_max from PSUM."""
    nc = bass.Bass("TRN2", target_bir_lowering=False, debug=False)

    qsT = nc.dram_tensor("qsT", [D, QROWS], F32, kind="ExternalInput").ap()
    psT = nc.dram_tensor("psT", [D, DOCS_PER_CORE * TD], F32,
                         kind="ExternalInput").ap()
    ones = nc.dram_tensor("ones", [128, QPG], F32, kind="ExternalInput").ap()
    out = nc.dram_tensor("out", [NQ, DOCS_PER_CORE], F32,
                         kind="ExternalOutput").ap()

    with tile.TileContext(nc) as tc, ExitStack() as ctx:
        const = ctx.enter_context(tc.tile_pool(name="const", bufs=1))
        psum = ctx.enter_context(tc.tile_pool(name="psum", bufs=3, space="PSUM"))
        psum_fin = ctx.enter_context(
            tc.tile_pool(name="psum_fin", bufs=1, space="PSUM"))

        qsT_sb = const.tile([D, QROWS], F32)
        nc.sync.dma_start(qsT_sb[:], qsT[:])
        ones_sb = const.tile([128, QPG], F32)
        nc.sync.dma_start(ones_sb[:], ones[:])
        psT_sb = const.tile([D, DOCS_PER_CORE * TD], F32)
        for dloc in range(DOCS_PER_CORE):
            sl = slice(dloc * TD, (dloc + 1) * TD)
            nc.sync.dma_start(psT_sb[:, sl], psT[:, sl])

        maxcols = const.tile([128, MG * DOCS_PER_CORE], F32)

        for dloc in range(DOCS_PER_CORE):
            for mg in range(MG):
                pt = psum.tile([128, TD], F32)
                lhsT = qsT_sb[:, mg * 128:(mg + 1) * 128]
                for h in range(TD // 512):
                    nc.tensor.matmul(
                        pt[:, h * 512:(h + 1) * 512],
                        lhsT=lhsT,
                        rhs=psT_sb[:, dloc * TD + h * 512:
                                   dloc * TD + (h + 1) * 512],
                        start=True, stop=True,
                    )
                col = mg * DOCS_PER_CORE + dloc
                nc.vector.reduce_max(
                    maxcols[:, col:col + 1], pt[:],
                    axis=mybir.AxisListType.X)

        fin = psum_fin.tile([QPG, MG * DOCS_PER_CORE], F32)
        nc.tensor.matmul(fin[:], lhsT=ones_sb[:], rhs=maxcols[:],
                         start=True, stop=True)
        out_sb = const.tile([QPG, MG * DOCS_PER_CORE], F32)
        nc.vector.tensor_copy(out_sb[:], fin[:])

        out_r = out.rearrange("(mg q) d -> q mg d", q=QPG)
        src = out_sb[:].rearrange("q (mg d) -> q mg d", d=DOCS_PER_CORE)
        nc.sync.dma_start(out_r, src)

    return nc


_NC_CACHE = {}


def _get_nc(mode=MODE, for_sim=False):
    # The wait-split pass breaks CoreSim's scheduler bookkeeping, so sim
    # uses an unsplit build; hardware needs the split to pass walrus.
    key = (mode, for_sim)
    if key not in _NC_CACHE:
        nc = (_build_pair_module() if mode == "pair"
              else _build_direct_module())
        if not for_sim:
            _split_multi_waits(nc)
        _NC_CACHE[key] = nc
    return _NC_CACHE[key]


def _ones_blockdiag():
    ones = np.zeros((128, QPG), dtype=np.float32)
    for q in range(QPG):
        ones[q * TQ:(q + 1) * TQ, q] = 1.0
    return ones


def _make_in_maps(qs, ps, mode=MODE):
    qs = np.ascontiguousarray(np.asarray(qs), dtype=np.float32)
    ps = np.ascontiguousarray(np.asarray(ps), dtype=np.float32)
    assert qs.shape == (NQ, TQ, D) and ps.shape == (ND, TD, D)

    in_maps = []
    if mode == "pair":
        qsT = np.ascontiguousarray(
            qs.reshape(QROWS, D).T.astype(np.float16))          # [128, 2048]
        pe = ps[:, 0::2, :]
        po = ps[:, 1::2, :]
        pplus = ((pe + po) * 0.5).astype(np.float16)            # [64,512,128]
        pminus = ((pe - po) * 0.5).astype(np.float16)
        for k in range(N_CORES):
            sh = slice(k * DOCS_PER_CORE, (k + 1) * DOCS_PER_CORE)
            pP = np.ascontiguousarray(
                pplus[sh].reshape(DOCS_PER_CORE * NPAIR, D).T)   # [128, 4096]
            pM = np.ascontiguousarray(
                pminus[sh].reshape(DOCS_PER_CORE * NPAIR, D).T)
            in_maps.append({"qsT": qsT, "psP": pP, "psM": pM})
    else:
        qsT = np.ascontiguousarray(qs.reshape(QROWS, D).T)      # [128, 2048]
        ones = _ones_blockdiag()
        for k in range(N_CORES):
            shard = ps[k * DOCS_PER_CORE:(k + 1) * DOCS_PER_CORE]
            psTk = np.ascontiguousarray(
                shard.reshape(DOCS_PER_CORE * TD, D).T)
            in_maps.append({"qsT": qsT, "psT": psTk, "ones": ones})
    return in_maps


def _gather(results, mode=MODE):
    if mode == "pair":
        # out[q, half, mg, dl] -> scores[(mg*4 + q), half*4 + dl]
        cols = []
        for k in range(N_CORES):
            o = results[k]["out"]           # [4, 2, 16, 4]
            cols.append(o.transpose(2, 0, 1, 3).reshape(NQ, DOCS_PER_CORE))
        return np.concatenate(cols, axis=1)
    return np.concatenate(
        [results[k]["out"] for k in range(N_CORES)], axis=1)


def kernel(qs, ps):
    nc = _get_nc()
    in_maps = _make_in_maps(qs, ps)
    res = bass_utils.run_bass_kernel_spmd(
        nc, in_maps, core_ids=list(range(N_CORES)))
    return _gather(res.results)


def kernel_timed(qs, ps, trace_cores=None):
    """Run with NTFF tracing; returns (scores, BassKernelResults)."""
    nc = _get_nc()
    in_maps = _make_in_maps(qs, ps)
    res = bass_utils.run_bass_kernel_spmd(
        nc, in_maps, core_ids=list(range(N_CORES)), trace=True,
        trace_cores=trace_cores)
    return _gather(res.results), res


# revision 19
# speedup vs baseline: 1.1837x; 1.1837x over previous
"""ColBERT MaxSim kernel for 8 Trainium2 NeuronCores.

scores[b, c] = sum_n max_s (qs[b, n, :] . ps[c, s, :])
  qs: (64, 32, 128) f32, ps: (64, 1024, 128) f32 -> scores: (64, 64) f32

Sharding: docs (c) are sharded 8 per core; qs is replicated. Each core
computes its (64, 8) score tile; the host concatenates along c. This puts
only ~2.5 MiB of DMA on each core (vs 33 MiB for batch-sharding) while the
per-core compute volume is identical.

Per-core dataflow (mode "pair", the default):
  - fp32 matmul on TRN2 streams at 2 cyc/col x 2 passes = 4x slower than
    16-bit, so inputs are cast to fp16 on the host (10 mantissa bits;
    measured end-to-end rel err ~2.4e-5 vs the fp32 reference).
  - Doc tokens are combined in PAIRS on the host: P+ = (P_even+P_odd)/2,
    P- = (P_even-P_odd)/2. Using max(a,b) = (a+b)/2 + |a-b|/2:
      PE:  S = Q.P+  (one PSUM bank),  D = Q.P-  (second bank)
      ACT: A = |D|   (pointwise Abs, PSUM->SBUF fp16, the only non-DVE
                      engine that can read PSUM)
      PE:  S += I.A  (accumulating identity matmul, start=False) — S's
                      bank now holds the 512 per-pair maxes
      DVE: reduce_max over [128, 2, 512] (two docs per op) -> maxcols
    This halves the VectorE reduce volume. The steady state is a genuine
    three-way fixed point at ~1.39us per 2-doc iteration: ACT is busy
    1.40us (2 ABS at (512+352)/1.2GHz), PE 1.32us (6 matmuls, LDWEIGHTS
    hidden by the reorder window), and the S-bank round trip
    (fill -> abs -> ident -> reduce, ~2.8us over 2 PSUM bufs) also lands
    at 1.4us. Batching/reordering any one leg pushes another past it, so
    the remaining headroom is all in startup and tail:
      * first DMA chunks are per-doc and split across both HWDGE queues
        (sync + scalar) so the first real matmul can start as soon as
        qsT-mg01 + psP/psM-doc0 land (~290 KiB critical bytes);
      * `ones` (block-diag) and `ident` are synthesized on-chip (gpsimd
        memset / affine_select), keeping the DMA queues clear;
      * the final token-sum matmul is split in half: docs 0-3 are summed
        and DMA'd out mid-kernel (hiding that DMA's ~1.5us latency), and
        the output DRAM layout is [q, half, mg, d] so each half is one
        contiguous 1 KiB burst instead of 65 strided 32 B packets.
  - Token-sum over each query's 32 rows via small fp16 matmuls with a
    block-diagonal ones matrix: out[4, 16, 4] = onesT.T @ maxcols-half.

Mode "direct" (env KERNEL_MODE=direct) is the exact-fp32 fallback:
fp32 matmuls + VectorE reduce_max straight from PSUM (~2.6x slower).
"""

import os
import sys
from contextlib import ExitStack

import numpy as np

sys.path.insert(0, "/opt/trn_rl_repo")
sys.path.insert(0, "/opt/trn_rl_repo/concourse")

import bass_rust
import concourse.bass as bass
import concourse.mybir as mybir
import concourse.tile as tile
from concourse import bass_utils

# Problem shape (hardcoded per contract)
N_CORES = 8
NQ, TQ, D = 64, 32, 128          # queries, query tokens, dim
ND, TD = 64, 1024                # docs, doc tokens
DOCS_PER_CORE = ND // N_CORES    # 8
QROWS = NQ * TQ                  # 2048 query-token rows
MG = QROWS // 128                # 16 M-groups of 128 rows
QPG = 128 // TQ                  # 4 queries per M-group
NPAIR = TD // 2                  # 512 token pairs per doc
HALF_D = DOCS_PER_CORE // 2      # 4 docs per output half

F32 = mybir.dt.float32
F16 = mybir.dt.float16

MODE = os.environ.get("KERNEL_MODE", "pair")


def _split_multi_waits(nc):
    """This walrus build rejects >1 embedded sync wait per instruction
    ("Too many sync wait commands"). Split extras onto single-wait NoOps
    inserted just before the instruction on the same engine — semantically
    identical (per-engine program order is preserved)."""
    n_split = 0
    for fn in nc.m.functions:
        for blk in fn.blocks:
            out = []
            for ins in blk.instructions:
                si = ins.sync_info
                waits = list(si.on_wait) if si and si.on_wait else []
                if len(waits) > 1:
                    for j, w in enumerate(waits[:-1]):
                        nop = mybir.InstNoOp(
                            name=f"{ins.name}_sw{j}", ins=[], outs=[])
                        nop.engine = ins.engine
                        nop.sync_info = bass_rust.SyncInfo(
                            on_wait=[w], on_update=[])
                        out.append(nop)
                    ins.sync_info = bass_rust.SyncInfo(
                        on_wait=[waits[-1]], on_update=list(si.on_update))
                    n_split += 1
                out.append(ins)
            blk.instructions = out
    return n_split


def _build_pair_module():
    nc = bass.Bass("TRN2", target_bir_lowering=False, debug=False)

    qsT = nc.dram_tensor("qsT", [D, QROWS], F16, kind="ExternalInput").ap()
    psP = nc.dram_tensor("psP", [D, DOCS_PER_CORE * NPAIR], F16,
                         kind="ExternalInput").ap()
    psM = nc.dram_tensor("psM", [D, DOCS_PER_CORE * NPAIR], F16,
                         kind="ExternalInput").ap()
    # out[q, half, mg, dl] = scores[(mg*4 + q), half*4 + dl] — each half is
    # a contiguous 1 KiB per-partition burst for the output DMA.
    out = nc.dram_tensor("out", [QPG, 2, MG, HALF_D], F32,
                         kind="ExternalOutput").ap()

    with tile.TileContext(nc) as tc, ExitStack() as ctx:
        const = ctx.enter_context(tc.tile_pool(name="const", bufs=1))
        stage = ctx.enter_context(tc.tile_pool(name="stage", bufs=10))
        # PSUM plan (8 banks): 2 × 2-bank S tiles + 4 × 1-bank D tiles.
        # (3 S bufs + 2 D bufs measured 18% SLOWER: with only two D banks
        # every D matmul chains directly onto the ABS two slots back, and
        # the pipeline locks at ~1.58us/iter with ~830ns PE waits.)
        psumS = ctx.enter_context(
            tc.tile_pool(name="psumS", bufs=2, space="PSUM"))
        psumD = ctx.enter_context(
            tc.tile_pool(name="psumD", bufs=4, space="PSUM"))

        # DMA issue costs ~650ns per dma_start on a sequencer; data lands
        # ~4us after issue (fixed DMA round trip under 8-core contention).
        # Critical-path bytes are minimized: the first real matmul needs
        # only qsT mg0 (32 KiB, sync) + psP doc0 (128 KiB, sync) +
        # psM doc0 (128 KiB, scalar). Everything else queues behind.
        qsT_sb = const.tile([D, QROWS], F16)
        psP_sb = const.tile([D, DOCS_PER_CORE * NPAIR], F16)
        psM_sb = const.tile([D, DOCS_PER_CORE * NPAIR], F16)
        # The first descriptor on each queue pays ~2.5us of fixed DMA
        # startup latency; everything behind it pipelines at full rate.
        # So the first chunks are as small as a whole consumer needs:
        # qsT M-group 0 alone (32 KB) unblocks the gated warmups and the
        # first S matmul ~0.5us earlier than a 64 KB chunk would.
        q0 = 128        # qsT M-group 0 (32 KB)
        c0 = 2 * NPAIR  # docs 0-1 of psP/psM (256 KB), split per-doc
        nc.sync.dma_start(qsT_sb[:, 0:q0], qsT[:, 0:q0])
        nc.scalar.dma_start(psM_sb[:, 0:NPAIR], psM[:, 0:NPAIR])
        nc.sync.dma_start(psP_sb[:, 0:NPAIR], psP[:, 0:NPAIR])
        nc.scalar.dma_start(psM_sb[:, NPAIR:c0], psM[:, NPAIR:c0])
        nc.sync.dma_start(psP_sb[:, NPAIR:c0], psP[:, NPAIR:c0])
        nc.sync.dma_start(qsT_sb[:, q0:2 * q0], qsT[:, q0:2 * q0])
        # Prefetch the Abs ACT table set (~2.7us TABLE_LOAD + drain) NOW —
        # emitted here it overlaps the initial DMA transfers instead of
        # gating the first real abs.
        warm = stage.tile([1, 2], F16, tag="warm")
        nc.gpsimd.memset(warm[:], 0.0)
        warm2 = stage.tile([1, 2], F16, tag="warm2")
        nc.scalar.activation(warm2[:], warm[:],
                             mybir.ActivationFunctionType.Abs)
        # qsT M-groups 2-3 go as their own small chunk ahead of the 384 KiB
        # remainder: iters 0-1 run off mg0-1 while this lands (~14us), so
        # M-group 2 no longer stalls ~1.3us on the big chunk (~17us).
        nc.sync.dma_start(qsT_sb[:, 2 * q0:4 * q0], qsT[:, 2 * q0:4 * q0])
        nc.sync.dma_start(qsT_sb[:, 4 * q0:], qsT[:, 4 * q0:])
        nc.scalar.dma_start(psM_sb[:, c0:], psM[:, c0:])
        nc.sync.dma_start(psP_sb[:, c0:], psP[:, c0:])

        # ident / ones are synthesized on-chip (gpsimd is otherwise idle),
        # keeping the HWDGE queues free for the real input tensors.
        ident_src = const.tile([128, 128], F16)
        nc.gpsimd.memset(ident_src[:], 1.0)
        ident_sb = const.tile([128, 128], F16)
        nc.gpsimd.affine_select(
            ident_sb[:], ident_src[:], pattern=[[-1, 128]],
            compare_op=mybir.AluOpType.is_equal, fill=0.0,
            base=0, channel_multiplier=1)
        ones_sb = const.tile([128, QPG], F16)
        nc.gpsimd.memset(ones_sb[:], 0.0)
        for q in range(QPG):
            nc.gpsimd.memset(ones_sb[q * TQ:(q + 1) * TQ, q:q + 1], 1.0)

        # HAM warmup: the PE is otherwise idle from the end of the NEFF
        # preamble (~7.5us) until the first DMA chunks land (~12us), and the
        # HAM clock gate needs ~3.4us of sustained PE activity to lift the
        # throttle from 1.2 to 2.4 GHz. A burst of matmuls on an
        # uninitialized (never-read) tile fills the activity window for
        # free, so the real matmul stream starts warm. 8 ungated (~3.4us
        # cold) + 2 gated on the first qsT chunk bridge the remaining gap
        # on cores whose DMA lands late (8-core HBM contention skews
        # arrivals by ~2.5us).
        garbage = const.tile([128, NPAIR], F16)
        nc.gpsimd.memset(garbage[:], 0.0)
        for _ in range(8):
            wt = psumD.tile([128, NPAIR], F32, tag="d")
            nc.tensor.matmul(wt[:], lhsT=garbage[:, 0:128], rhs=garbage[:],
                             start=True, stop=True)
        # The gated pair is narrow (N=128): their job is a blip of PE
        # activity once the first qsT chunk lands, not work — at N=512
        # they queue 0.85us of matmul ahead of the first real S matmul
        # (S0 would start at ~13.15us instead of data-ready ~12.9us).
        # Idle-window/HAM protection is unchanged: it depends on when the
        # bursts run, not their length.
        for _ in range(2):
            wt = psumD.tile([128, NPAIR], F32, tag="d")
            nc.tensor.matmul(wt[:, 0:128], lhsT=qsT_sb[:, 0:128],
                             rhs=garbage[:, 0:128], start=True, stop=True)

        # maxcols[p, mg, dloc] = max over doc dloc's tokens for row p of mg
        maxcols = const.tile([128, MG, DOCS_PER_CORE], F16)
        out_sb = const.tile([QPG, 2, MG * HALF_D], F32)

        def emit_fin(half):
            # Token-sum for docs [half*4, half*4+4): fin[q, mg, dl] =
            # sum_p ones[p, q] * maxcols[p, mg, half*4+dl]. Uses a psumD
            # rotation slot (the fin tile is tiny and its deps are long
            # done, so the extra rotation costs ~0.4us of PE time at most).
            finh = psumD.tile([QPG, MG * HALF_D], F32, tag="d")
            src = maxcols[:, :, half * HALF_D:(half + 1) * HALF_D]
            nc.tensor.matmul(finh[:].rearrange("q (mg d) -> q mg d",
                                               d=HALF_D),
                             lhsT=ones_sb[:], rhs=src,
                             start=True, stop=True)
            nc.vector.tensor_copy(out_sb[:, half], finh[:])
            nc.sync.dma_start(
                out.rearrange("q h mg d -> q h (mg d)")[:, half],
                out_sb[:, half])

        for dp in range(DOCS_PER_CORE // 2):
            for mg in range(MG):
                lhsT = qsT_sb[:, mg * 128:(mg + 1) * 128]
                # Two docs (2*dp, 2*dp+1) share one 2-bank S tile so the
                # VectorE reduce below covers both in a single instruction.
                s2 = psumS.tile([128, 2 * NPAIR], F32, tag="s")
                for h in range(2):
                    dloc = 2 * dp + h
                    sl = slice(dloc * NPAIR, (dloc + 1) * NPAIR)
                    sb = s2[:, h * NPAIR:(h + 1) * NPAIR]
                    # S = Q.P+ (accumulation group stays open)
                    nc.tensor.matmul(sb, lhsT=lhsT,
                                     rhs=psP_sb[:, sl], start=True,
                                     stop=False, skip_group_check=True)
                    # D = Q.P- (separate pool: released after ACT)
                    dt = psumD.tile([128, NPAIR], F32, tag="d")
                    nc.tensor.matmul(dt[:], lhsT=lhsT,
                                     rhs=psM_sb[:, sl], start=True,
                                     stop=True, skip_group_check=True)
                    # A = |D| (fp16, SBUF) on ScalarE — the 2nd PSUM reader
                    a = stage.tile([128, NPAIR], F16)
                    nc.scalar.activation(a[:], dt[:],
                                         mybir.ActivationFunctionType.Abs)
                    # S += I.A  -> S half now holds the 512 per-pair maxes
                    nc.tensor.matmul(sb, lhsT=ident_sb[:],
                                     rhs=a[:], start=False, stop=True,
                                     skip_group_check=True)
                # One [128, 2, 512] reduce for both docs: two per-doc
                # [1, 512] reduces measured 693ns each (DVE 1386/iter +
                # sems -> 1572ns/iter pace, 118.6us total) vs 1224ns
                # batched — the per-instruction overhead swamps the
                # earlier S-bank release.
                nc.vector.reduce_max(
                    maxcols[:, mg, 2 * dp:2 * dp + 2],
                    s2[:].rearrange("p (h n) -> p h n", h=2),
                    axis=mybir.AxisListType.X)
                if dp == 2 and mg == 4:
                    # Docs 0-3 finished ~1.4us/iter * 5 iters ago; their
                    # token-sum + output DMA hide entirely mid-kernel.
                    emit_fin(0)
        emit_fin(1)

    return nc


def _build_direct_module():
    """Exact-fp32 fallback: fp32 matmuls + DVE reduce_max from PSUM."""
    nc = bass.Bass("TRN2", target_bir_lowering=False, debug=False)

    qsT = nc.dram_tensor("qsT", [D, QROWS], F32, kind="ExternalInput").ap()
    psT = nc.dram_tensor("psT", [D, DOCS_PER_CORE * TD], F32,
                         kind="ExternalInput").ap()
    ones = nc.dram_tensor("ones", [128, QPG], F32, kind="ExternalInput").ap()
    out = nc.dram_tensor("out", [NQ, DOCS_PER_CORE], F32,
                         kind="ExternalOutput").ap()

    with tile.TileContext(nc) as tc, ExitStack() as ctx:
        const = ctx.enter_context(tc.tile_pool(name="const", bufs=1))
        psum = ctx.enter_context(tc.tile_pool(name="psum", bufs=3, space="PSUM"))
        psum_fin = ctx.enter_context(
            tc.tile_pool(name="psum_fin", bufs=1, space="PSUM"))

        qsT_sb = const.tile([D, QROWS], F32)
        nc.sync.dma_start(qsT_sb[:], qsT[:])
        ones_sb = const.tile([128, QPG], F32)
        nc.sync.dma_start(ones_sb[:], ones[:])
        psT_sb = const.tile([D, DOCS_PER_CORE * TD], F32)
        for dloc in range(DOCS_PER_CORE):
            sl = slice(dloc * TD, (dloc + 1) * TD)
            nc.sync.dma_start(psT_sb[:, sl], psT[:, sl])

        maxcols = const.tile([128, MG * DOCS_PER_CORE], F32)

        for dloc in range(DOCS_PER_CORE):
            for mg in range(MG):
                pt = psum.tile([128, TD], F32)
                lhsT = qsT_sb[:, mg * 128:(mg + 1) * 128]
                for h in range(TD // 512):
                    nc.tensor.matmul(
                        pt[:, h * 512:(h + 1) * 512],
                        lhsT=lhsT,
                        rhs=psT_sb[:, dloc * TD + h * 512:
                                   dloc * TD + (h + 1) * 512],
                        start=True, stop=True,
                    )
                col = mg * DOCS_PER_CORE + dloc
                nc.vector.reduce_max(
                    maxcols[:, col:col + 1], pt[:],
                    axis=mybir.AxisListType.X)

        fin = psum_fin.tile([QPG, MG * DOCS_PER_CORE], F32)
        nc.tensor.matmul(fin[:], lhsT=ones_sb[:], rhs=maxcols[:],
                         start=True, stop=True)
        out_sb = const.tile([QPG, MG * DOCS_PER_CORE], F32)
        nc.vector.tensor_copy(out_sb[:], fin[:])

        out_r = out.rearrange("(mg q) d -> q mg d", q=QPG)
        src = out_sb[:].rearrange("q (mg d) -> q mg d", d=DOCS_PER_CORE)
        nc.sync.dma_start(out_r, src)

    return nc


_NC_CACHE = {}


def _get_nc(mode=MODE, for_sim=False):
    # The wait-split pass breaks CoreSim's scheduler bookkeeping, so sim
    # uses an unsplit build; hardware needs the split to pass walrus.
    key = (mode, for_sim)
    if key not in _NC_CACHE:
        nc = (_build_pair_module() if mode == "pair"
              else _build_direct_module())
        if not for_sim:
            _split_multi_waits(nc)
        _NC_CACHE[key] = nc
    return _NC_CACHE[key]


def _ones_blockdiag():
    ones = np.zeros((128, QPG), dtype=np.float32)
    for q in range(QPG):
        ones[q * TQ:(q + 1) * TQ, q] = 1.0
    return ones


def _make_in_maps(qs, ps, mode=MODE):
    qs = np.ascontiguousarray(np.asarray(qs), dtype=np.float32)
    ps = np.ascontiguousarray(np.asarray(ps), dtype=np.float32)
    assert qs.shape == (NQ, TQ, D) and ps.shape == (ND, TD, D)

    in_maps = []
    if mode == "pair":
        qsT = np.ascontiguousarray(
            qs.reshape(QROWS, D).T.astype(np.float16))          # [128, 2048]
        pe = ps[:, 0::2, :]
        po = ps[:, 1::2, :]
        pplus = ((pe + po) * 0.5).astype(np.float16)            # [64,512,128]
        pminus = ((pe - po) * 0.5).astype(np.float16)
        for k in range(N_CORES):
            sh = slice(k * DOCS_PER_CORE, (k + 1) * DOCS_PER_CORE)
            pP = np.ascontiguousarray(
                pplus[sh].reshape(DOCS_PER_CORE * NPAIR, D).T)   # [128, 4096]
            pM = np.ascontiguousarray(
                pminus[sh].reshape(DOCS_PER_CORE * NPAIR, D).T)
            in_maps.append({"qsT": qsT, "psP": pP, "psM": pM})
    else:
        qsT = np.ascontiguousarray(qs.reshape(QROWS, D).T)      # [128, 2048]
        ones = _ones_blockdiag()
        for k in range(N_CORES):
            shard = ps[k * DOCS_PER_CORE:(k + 1) * DOCS_PER_CORE]
            psTk = np.ascontiguousarray(
                shard.reshape(DOCS_PER_CORE * TD, D).T)
            in_maps.append({"qsT": qsT, "psT": psTk, "ones": ones})
    return in_maps


def _gather(results, mode=MODE):
    if mode == "pair":
        # out[q, half, mg, dl] -> scores[(mg*4 + q), half*4 + dl]
        cols = []
        for k in range(N_CORES):
            o = results[k]["out"]           # [4, 2, 16, 4]
            cols.append(o.transpose(2, 0, 1, 3).reshape(NQ, DOCS_PER_CORE))
        return np.concatenate(cols, axis=1)
    return np.concatenate(
        [results[k]["out"] for k in range(N_CORES)], axis=1)


def kernel(qs, ps):
    nc = _get_nc()
    in_maps = _make_in_maps(qs, ps)
    res = bass_utils.run_bass_kernel_spmd(
        nc, in_maps, core_ids=list(range(N_CORES)))
    return _gather(res.results)


def kernel_timed(qs, ps, trace_cores=None):
    """Run with NTFF tracing; returns (scores, BassKernelResults)."""
    nc = _get_nc()
    in_maps = _make_in_maps(qs, ps)
    res = bass_utils.run_bass_kernel_spmd(
        nc, in_maps, core_ids=list(range(N_CORES)), trace=True,
        trace_cores=trace_cores)
    return _gather(res.results), res
